# revision 41
# baseline (speedup 1.0000x reference)
"""Trainium2 Bass kernel for causal FFT convolution (nn_CausalConvolution).

y = irfft(rfft(bf16(x), 2T) * rfft(h, 2T))[..., :T],  x,h: (8, 64, 65536) fp32.

Identity used: with z = bf16(x) + i*h,  y = Im(iFFT(FFT_2T(z)^2)) / 2.
One complex forward + one complex inverse FFT per channel, N = 131072,
decomposed as radix (128, 128, 8) matmul stages on the PE with PE-transposes
between stages. 512 channels sharded 64-per-core across 8 NeuronCores (pure
data parallelism).

v3: bf16 intermediates + stacked [x|h] S1 + engine-balanced evacuations
(Act: f32 PSUM, DVE: bf16 PSUM + twiddle mults, GpSimd: wide SBUF adds) +
skewed software pipeline: at step t the kernel emits stage s of block t-s,
so every instruction's inputs were produced a full step earlier and the PE
never stalls at stage boundaries (keeps the PE p-state at full clock).

Self-contained: shapes/sharding hardcoded; tables computed with numpy here.
"""
import numpy as np
import ml_dtypes
from contextlib import ExitStack

import concourse.bass as bass
import concourse.bacc as bacc
import concourse.tile as tile
import concourse.mybir as mybir
from concourse.bass_utils import run_bass_kernel_spmd

F32 = mybir.dt.float32
BF16 = mybir.dt.bfloat16
MUL = mybir.AluOpType.mult
ADD = mybir.AluOpType.add
SUB = mybir.AluOpType.subtract

Bsz, Csz, T = 8, 64, 65536
NFFT = 2 * T
NCORES = 8
CPC = (Bsz * Csz) // NCORES          # 64 channels per core
NBLK = CPC // 2                      # 2 channels per block

_Wc = lambda M, E: np.exp(-2j * np.pi * E / M)


def _gen_tables():
    F128 = _Wc(128, np.outer(np.arange(128), np.arange(128)))
    F8 = _Wc(8, np.outer(np.arange(8), np.arange(8)))
    W1024bd = _Wc(1024, np.outer(np.arange(8), np.arange(128)))     # [b, d]
    TW1_cab = _Wc(NFFT, (8 * np.arange(128)[None, :, None]
                         + np.arange(8)[None, None, :])
                  * np.arange(128)[:, None, None])                  # [c, a, b]

    f32 = lambda v: np.ascontiguousarray(v, dtype=np.float32)
    t = {}
    # ---- S1 packed (contract [u<64 | u<64] = [xq | h] stacked): lhsT[128, c]
    # pre = s1_re^T xq + s1_imn^T h ; pim = s1_im^T xq + s1_re^T h
    t["s1A"] = f32(np.concatenate([F128[:64].real, -F128[:64].imag]))
    t["s1B"] = f32(np.concatenate([F128[:64].imag, F128[:64].real]))

    # ---- packed [n,128,128] bf16 stationaries ----
    mats = []
    idx = {}

    def put(name, m):
        idx[name] = len(mats)
        mats.append(f32(m))

    S2 = F128[None, :, :] * W1024bd[:, None, :]                     # [b, a, d]
    for b in range(8):
        put(f"s2_re{b}", S2[b].real)
        put(f"s2_im{b}", S2[b].imag)
        put(f"s2_imn{b}", -S2[b].imag)
    S3 = np.zeros((128, 128), np.complex128)
    for b in range(8):
        for e in range(8):
            for c16 in range(16):
                S3[c16 * 8 + b, c16 * 8 + e] = F8[b, e]
    put("s3_re", S3.real)
    put("s3_im", S3.imag)
    put("s3_imn", -S3.imag)
    S3p = np.zeros((128, 128), np.complex128)
    for e in range(8):
        for bp in range(8):
            for c16 in range(16):
                S3p[c16 * 8 + e, c16 * 8 + bp] = np.conj(F8[bp, e])
    put("s3p_re", S3p.real)          # multiplies rhs_re -> psum_re
    put("s3p_im", S3p.imag)          # rhs_re -> psum_im
    put("s3p_imn2", -2 * S3p.imag)   # rhs_im -> psum_re (x2: Sim stored halved)
    put("s3p_re2", 2 * S3p.real)     # rhs_im -> psum_im
    S2p = np.conj(S2).transpose(0, 2, 1)                            # [bp, d, a]
    for b in range(8):
        put(f"s2p_re{b}", S2p[b].real)
        put(f"s2p_im{b}", S2p[b].imag)
        put(f"s2p_imn{b}", -S2p[b].imag)
    put("ident", np.eye(128))
    t["st128"] = np.stack(mats)                                     # [n,128,128]
    t["st128_idx"] = idx

    # ---- S1' (contract c, produce u<64, imag plane only, scale 1/(2N)) ----
    S1p = np.conj(F128).T[:, :64] / (2.0 * NFFT)                    # [c, u]
    t["s1p_re"] = f32(S1p.real)
    t["s1p_im"] = f32(S1p.imag)

    # ---- twiddle tables (2048-wide: TW1 duplicated across the 2 channels) ----
    m_ord = np.tile(TW1_cab.reshape(128, 1024), (1, 2))             # [c, ch*1024+m]
    tw1p = np.zeros((128, 2048), np.complex128)                     # [a, bp*256+ch*128+c]
    for bp in range(8):
        for ch in range(2):
            sl = slice(bp * 256 + ch * 128, bp * 256 + ch * 128 + 128)
            tw1p[:, sl] = np.conj(TW1_cab[:, :, bp]).T
    t["tw"] = np.concatenate(
        [f32(m_ord.real), f32(m_ord.imag),
         f32(tw1p.real), f32(tw1p.imag)], axis=1)                   # [128, 8192]
    return t


def _build(n_blocks=NBLK, debug=False):
    tabs = _gen_tables()
    nc = bacc.Bacc("TRN2", target_bir_lowering=False, debug=False)

    x_d = nc.dram_tensor("x_in", [CPC, 64, 1024], BF16, kind="ExternalInput").ap()
    h_d = nc.dram_tensor("h_in", [CPC, 64, 1024], BF16, kind="ExternalInput").ap()
    nst = tabs["st128"].shape[0]
    st_d = nc.dram_tensor("st_in", [nst, 128, 128], BF16, kind="ExternalInput").ap()
    s1_d = nc.dram_tensor("s1_in", [2, 128, 128], BF16, kind="ExternalInput").ap()
    s1p_d = nc.dram_tensor("s1p_in", [2, 128, 64], BF16, kind="ExternalInput").ap()
    tw_d = nc.dram_tensor("tw_in", [128, 8192], BF16, kind="ExternalInput").ap()
    y_d = nc.dram_tensor("y_out", [CPC, 64, 1024], F32, kind="ExternalOutput").ap()

    with tile.TileContext(nc) as tc, ExitStack() as ctx:
        const = ctx.enter_context(tc.tile_pool(name="const", bufs=1))
        data = ctx.enter_context(tc.tile_pool(name="io", bufs=3))
        stage = ctx.enter_context(tc.tile_pool(name="stage", bufs=6))
        stageB = ctx.enter_context(tc.tile_pool(name="stageB", bufs=5))
        tmp = ctx.enter_context(tc.tile_pool(name="tmp", bufs=3))
        tmp5 = ctx.enter_context(tc.tile_pool(name="tmp5", bufs=2))
        psum = ctx.enter_context(tc.tile_pool(name="psum", bufs=4, space="PSUM"))

        # ---- load constant tables once ----
        st = const.tile([128, nst * 128], BF16, tag="st")
        nc.sync.dma_start(
            st[:].rearrange("p (n c) -> p n c", n=nst),
            st_d.rearrange("n p c -> p n c"))
        s1t = const.tile([128, 2 * 128], BF16, tag="s1t")
        nc.sync.dma_start(s1t[:].rearrange("p (n c) -> p n c", n=2),
                          s1_d.rearrange("n p c -> p n c"))
        s1p = const.tile([128, 2 * 64], BF16, tag="s1p")
        nc.sync.dma_start(s1p[:].rearrange("p (n c) -> p n c", n=2),
                          s1p_d.rearrange("n p c -> p n c"))
        tw = const.tile([128, 8192], BF16, tag="tw")
        nc.sync.dma_start(tw[:], tw_d)

        sidx = tabs["st128_idx"]
        M = lambda name: st[:, sidx[name] * 128:(sidx[name] + 1) * 128]
        ident = M("ident")
        s1A, s1B = s1t[:, 0:128], s1t[:, 128:256]
        s1p_re, s1p_im = s1p[:, 0:64], s1p[:, 64:128]
        tw1_re, tw1_im = tw[:, 0:2048], tw[:, 2048:4096]
        tw1p_re, tw1p_im = tw[:, 4096:6144], tw[:, 6144:8192]

        def cmm(pre, pim, mrr, mir, mri, mii, rre, rim, start, stop):
            """pre += mrr.T@rre + mir.T@rim ; pim += mri.T@rre + mii.T@rim"""
            nc.tensor.matmul(pre, mrr, rre, start=start, stop=False)
            nc.tensor.matmul(pre, mir, rim, start=False, stop=stop)
            nc.tensor.matmul(pim, mri, rre, start=start, stop=False)
            nc.tensor.matmul(pim, mii, rim, start=False, stop=stop)

        def pair():
            pr = psum.tile([128, 512], F32, tag="pr")
            pi = psum.tile([128, 512], F32, tag="pi")
            return pr, pi

        def pairT():
            pr = psum.tile([128, 512], BF16, tag="pr")
            pi = psum.tile([128, 512], BF16, tag="pi")
            return pr, pi

        live = {}                     # blk -> {name: tile}

        # ================= stage functions (one block each) =================
        def st_load(blk):
            ch0 = 2 * blk
            zq = data.tile([128, 2048], BF16, tag="zq")
            for ch in range(2):
                nc.sync.dma_start(zq[0:64, ch * 1024:(ch + 1) * 1024], x_d[ch0 + ch])
                nc.sync.dma_start(zq[64:128, ch * 1024:(ch + 1) * 1024], h_d[ch0 + ch])
            live[blk] = {"zq": zq}

        def st_s1(blk):
            # S1 + EV1(TW1): B1 [c x (ch*1024 + a*8 + b)]
            L = live[blk]
            zq = L.pop("zq")
            b1re = stage.tile([128, 2048], BF16, tag="pAre")
            b1im = stage.tile([128, 2048], BF16, tag="pAim")
            t1 = tmp.tile([128, 2048], BF16, tag="t1")
            t2 = tmp.tile([128, 2048], BF16, tag="t2")
            t3 = tmp.tile([128, 2048], BF16, tag="t3")
            t4 = tmp.tile([128, 2048], BF16, tag="t4")
            for ck in range(4):
                cs = slice(ck * 512, (ck + 1) * 512)
                pr, pi = pair()
                nc.tensor.matmul(pr[:], s1A, zq[:, cs], start=True, stop=True)
                nc.tensor.matmul(pi[:], s1B, zq[:, cs], start=True, stop=True)
                nc.vector.tensor_tensor(t1[:, cs], pr[:], tw1_re[:, cs], MUL)
                nc.vector.tensor_tensor(t2[:, cs], pi[:], tw1_im[:, cs], MUL)
                nc.vector.tensor_tensor(t3[:, cs], pr[:], tw1_im[:, cs], MUL)
                nc.vector.tensor_tensor(t4[:, cs], pi[:], tw1_re[:, cs], MUL)
            nc.gpsimd.tensor_tensor(b1re[:], t1[:], t2[:], SUB)
            nc.gpsimd.tensor_tensor(b1im[:], t3[:], t4[:], ADD)
            L["b1re"], L["b1im"] = b1re, b1im

        def st_tr1(blk):
            # TR1: B2 [a x (b*256 + ch*128 + c)]   (evac: DVE 2x)
            L = live[blk]
            b1re, b1im = L.pop("b1re"), L.pop("b1im")
            b2re = stageB.tile([128, 2048], BF16, tag="pBre")
            b2im = stageB.tile([128, 2048], BF16, tag="pBim")
            b1v_re = b1re[:].rearrange("p (ch a b) -> p ch a b", ch=2, a=128, b=8)
            b1v_im = b1im[:].rearrange("p (ch a b) -> p ch a b", ch=2, a=128, b=8)
            b2v_re = b2re[:].rearrange("p (b ch c) -> p b ch c", b=8, ch=2, c=128)
            b2v_im = b2im[:].rearrange("p (b ch c) -> p b ch c", b=8, ch=2, c=128)
            for ch in range(2):
                for hb in range(2):
                    pr, pi = pairT()
                    for j in range(4):
                        b = hb * 4 + j
                        s = slice(j * 128, (j + 1) * 128)
                        nc.tensor.transpose(pr[:, s], b1v_re[:, ch, :, b], ident)
                        nc.tensor.transpose(pi[:, s], b1v_im[:, ch, :, b], ident)
                    for ps, ov in ((pr, b2v_re), (pi, b2v_im)):
                        nc.vector.tensor_copy(
                            ov[:, hb * 4:(hb + 1) * 4, ch, :],
                            ps[:].rearrange("p (j c) -> p j c", j=4))
            L["b2re"], L["b2im"] = b2re, b2im

        def st_s2(blk):
            # S2: B3 [d x (ch*1024 + c*8 + b)]   (evac: Act)
            L = live[blk]
            b2re, b2im = L.pop("b2re"), L.pop("b2im")
            b3re = stage.tile([128, 2048], BF16, tag="pAre")
            b3im = stage.tile([128, 2048], BF16, tag="pAim")
            b3v_re = b3re[:].rearrange("p (ch c b) -> p ch c b", ch=2, c=128, b=8)
            b3v_im = b3im[:].rearrange("p (ch c b) -> p ch c b", ch=2, c=128, b=8)
            for hb in range(4):
                pr, pi = pair()
                for j in range(2):
                    b = hb * 2 + j
                    s = slice(j * 256, (j + 1) * 256)
                    rs = slice(b * 256, (b + 1) * 256)
                    cmm(pr[:, s], pi[:, s],
                        M(f"s2_re{b}"), M(f"s2_imn{b}"), M(f"s2_im{b}"), M(f"s2_re{b}"),
                        b2re[:, rs], b2im[:, rs], True, True)
                for ps, ov in ((pr, b3v_re), (pi, b3v_im)):
                    iv = ps[:].rearrange("p (j ch c) -> p j ch c", j=2, ch=2)
                    for ch in range(2):
                        nc.scalar.copy(
                            ov[:, ch, :, hb * 2:(hb + 1) * 2]
                            .rearrange("p c j -> p j c"),
                            iv[:, :, ch, :])
            L["b3re"], L["b3im"] = b3re, b3im

        def st_tr2(blk):
            # TR2: B4 [(c16*8+b) x (ch*1024 + chi*128 + d)]   (evac: Act)
            L = live[blk]
            b3re, b3im = L.pop("b3re"), L.pop("b3im")
            b4re = stageB.tile([128, 2048], BF16, tag="pBre")
            b4im = stageB.tile([128, 2048], BF16, tag="pBim")
            for ch in range(2):
                for hc in range(2):
                    pr, pi = pairT()
                    for j in range(4):
                        chi = hc * 4 + j
                        s = slice(j * 128, (j + 1) * 128)
                        src = slice(ch * 1024 + chi * 128, ch * 1024 + (chi + 1) * 128)
                        nc.tensor.transpose(pr[:, s], b3re[:, src], ident)
                        nc.tensor.transpose(pi[:, s], b3im[:, src], ident)
                    ds = slice(ch * 1024 + hc * 512, ch * 1024 + (hc + 1) * 512)
                    nc.vector.tensor_copy(b4re[:, ds], pr[:])
                    nc.vector.tensor_copy(b4im[:, ds], pi[:])
            L["b4re"], L["b4im"] = b4re, b4im

        def st_s3(blk):
            # S3 + EV5(square): B5 = (Sre, Sim/2)
            L = live[blk]
            b4re, b4im = L.pop("b4re"), L.pop("b4im")
            b5re = stage.tile([128, 2048], BF16, tag="pAre")
            b5im = stage.tile([128, 2048], BF16, tag="pAim")
            t5re = tmp5.tile([128, 2048], BF16, tag="t5re")
            t5im = tmp5.tile([128, 2048], BF16, tag="t5im")
            u1 = tmp.tile([128, 2048], BF16, tag="t1")
            u2 = tmp.tile([128, 2048], BF16, tag="t2")
            for ck in range(4):
                cs = slice(ck * 512, (ck + 1) * 512)
                pr, pi = pair()
                cmm(pr[:], pi[:], M("s3_re"), M("s3_imn"), M("s3_im"), M("s3_re"),
                    b4re[:, cs], b4im[:, cs], True, True)
                nc.scalar.copy(t5re[:, cs], pr[:])
                nc.scalar.copy(t5im[:, cs], pi[:])
            nc.vector.tensor_tensor(u1[:], t5re[:], t5re[:], MUL)
            nc.vector.tensor_tensor(u2[:], t5im[:], t5im[:], MUL)
            nc.gpsimd.tensor_tensor(b5re[:], u1[:], u2[:], SUB)
            nc.gpsimd.tensor_tensor(b5im[:], t5re[:], t5im[:], MUL)
            L["b5re"], L["b5im"] = b5re, b5im

        def st_s3p(blk):
            # S3': B6 [(c16*8+bp) x cols]   (evac: Act)
            L = live[blk]
            b5re, b5im = L.pop("b5re"), L.pop("b5im")
            b6re = stageB.tile([128, 2048], BF16, tag="pBre")
            b6im = stageB.tile([128, 2048], BF16, tag="pBim")
            for ck in range(4):
                cs = slice(ck * 512, (ck + 1) * 512)
                pr, pi = pair()
                cmm(pr[:], pi[:], M("s3p_re"), M("s3p_imn2"), M("s3p_im"), M("s3p_re2"),
                    b5re[:, cs], b5im[:, cs], True, True)
                nc.scalar.copy(b6re[:, cs], pr[:])
                nc.scalar.copy(b6im[:, cs], pi[:])
            L["b6re"], L["b6im"] = b6re, b6im

        def st_tr3(blk):
            # TR3: B7 [d x (ch, hc, chi4, c16, bp)] psum-natural (evac: DVE 2x)
            L = live[blk]
            b6re, b6im = L.pop("b6re"), L.pop("b6im")
            b7re = stage.tile([128, 2048], BF16, tag="pAre")
            b7im = stage.tile([128, 2048], BF16, tag="pAim")
            for ch in range(2):
                for hc in range(2):
                    pr, pi = pairT()
                    for j in range(4):
                        chi = hc * 4 + j
                        s = slice(j * 128, (j + 1) * 128)
                        src = slice(ch * 1024 + chi * 128, ch * 1024 + (chi + 1) * 128)
                        nc.tensor.transpose(pr[:, s], b6re[:, src], ident)
                        nc.tensor.transpose(pi[:, s], b6im[:, src], ident)
                    ds = slice((ch * 2 + hc) * 512, (ch * 2 + hc + 1) * 512)
                    nc.vector.tensor_copy(b7re[:, ds], pr[:])
                    nc.vector.tensor_copy(b7im[:, ds], pi[:])
            L["b7re"], L["b7im"] = b7re, b7im

        def st_s2p(blk):
            # S2' + EV8(TW1'): B8 [a x (bp*256 + ch*128 + c)]
            # rhs gathers the bp-strided columns out of psum-natural B7.
            L = live[blk]
            b7re, b7im = L.pop("b7re"), L.pop("b7im")
            b8re = stageB.tile([128, 2048], BF16, tag="pBre")
            b8im = stageB.tile([128, 2048], BF16, tag="pBim")
            b7g_re = b7re[:].rearrange("p (chhc chic16 bp) -> p bp chhc chic16",
                                       chhc=4, chic16=64, bp=8)
            b7g_im = b7im[:].rearrange("p (chhc chic16 bp) -> p bp chhc chic16",
                                       chhc=4, chic16=64, bp=8)
            t1 = tmp.tile([128, 2048], BF16, tag="t1")
            t2 = tmp.tile([128, 2048], BF16, tag="t2")
            t3 = tmp.tile([128, 2048], BF16, tag="t3")
            t4 = tmp.tile([128, 2048], BF16, tag="t4")
            for hb in range(4):
                pr, pi = pair()
                for j in range(2):
                    b = hb * 2 + j
                    s = slice(j * 256, (j + 1) * 256)
                    cmm(pr[:, s], pi[:, s],
                        M(f"s2p_re{b}"), M(f"s2p_imn{b}"), M(f"s2p_im{b}"), M(f"s2p_re{b}"),
                        b7g_re[:, b], b7g_im[:, b], True, True)
                cs = slice(hb * 512, (hb + 1) * 512)
                nc.vector.tensor_tensor(t1[:, cs], pr[:], tw1p_re[:, cs], MUL)
                nc.vector.tensor_tensor(t2[:, cs], pi[:], tw1p_im[:, cs], MUL)
                nc.vector.tensor_tensor(t3[:, cs], pr[:], tw1p_im[:, cs], MUL)
                nc.vector.tensor_tensor(t4[:, cs], pi[:], tw1p_re[:, cs], MUL)
            nc.gpsimd.tensor_tensor(b8re[:], t1[:], t2[:], SUB)
            nc.gpsimd.tensor_tensor(b8im[:], t3[:], t4[:], ADD)
            L["b8re"], L["b8im"] = b8re, b8im

        def st_tr4(blk):
            # TR4: B9 [c x (ch, hb, bpj4, a)] psum-natural (evac: DVE 2x)
            L = live[blk]
            b8re, b8im = L.pop("b8re"), L.pop("b8im")
            b9re = stage.tile([128, 2048], BF16, tag="pAre")
            b9im = stage.tile([128, 2048], BF16, tag="pAim")
            b8v_re = b8re[:].rearrange("p (bp ch c) -> p bp ch c", bp=8, ch=2, c=128)
            b8v_im = b8im[:].rearrange("p (bp ch c) -> p bp ch c", bp=8, ch=2, c=128)
            for ch in range(2):
                for hb in range(2):
                    pr, pi = pairT()
                    for j in range(4):
                        bp = hb * 4 + j
                        s = slice(j * 128, (j + 1) * 128)
                        nc.tensor.transpose(pr[:, s], b8v_re[:, bp, ch, :], ident)
                        nc.tensor.transpose(pi[:, s], b8v_im[:, bp, ch, :], ident)
                    ds = slice((ch * 2 + hb) * 512, (ch * 2 + hb + 1) * 512)
                    nc.vector.tensor_copy(b9re[:, ds], pr[:])
                    nc.vector.tensor_copy(b9im[:, ds], pi[:])
            L["b9re"], L["b9im"] = b9re, b9im

        def st_s1p(blk):
            # S1' + store: psum cols (bpj4, a128); yt scatters m = a*8+hb*4+j.
            ch0 = 2 * blk
            L = live.pop(blk)
            b9re, b9im = L["b9re"], L["b9im"]
            for ch in range(2):
                yt = data.tile([64, 1024], F32, tag="yt")
                ytv = yt[:].rearrange("u (a e) -> u a e", e=8)
                for hb in range(2):
                    p10 = psum.tile([64, 512], F32, tag="pr")
                    rs = slice((ch * 2 + hb) * 512, (ch * 2 + hb + 1) * 512)
                    nc.tensor.matmul(p10[:], s1p_im, b9re[:, rs], start=True, stop=False)
                    nc.tensor.matmul(p10[:], s1p_re, b9im[:, rs], start=False, stop=True)
                    nc.scalar.copy(
                        ytv[:, :, hb * 4:(hb + 1) * 4].rearrange("u a j -> u j a"),
                        p10[:].rearrange("u (j a) -> u j a", j=4))
                nc.sync.dma_start(y_d[ch0 + ch], yt[:])

        STAGES = [st_load, st_s1, st_tr1, st_s2, st_tr2, st_s3,
                  st_s3p, st_tr3, st_s2p, st_tr4, st_s1p]
        ns = len(STAGES)
        # Skewed pipeline: step t emits stage s of block t-s (deepest first).
        for t in range(n_blocks + ns - 1):
            for s in reversed(range(ns)):
                b = t - s
                if 0 <= b < n_blocks:
                    STAGES[s](b)

    nc.compile()
    return nc, tabs


_CACHE = {}


def _get(n_blocks=NBLK, debug=False):
    key = (n_blocks, debug)
    if key not in _CACHE:
        _CACHE[key] = _build(n_blocks, debug)
    return _CACHE[key]


def _in_maps(x, h, tabs):
    bf16 = ml_dtypes.bfloat16
    xf = np.ascontiguousarray(x, np.float32).reshape(Bsz * Csz, 65536)
    hf = np.ascontiguousarray(h, np.float32).reshape(Bsz * Csz, 65536)
    s1 = np.stack([tabs["s1A"], tabs["s1B"]]).astype(bf16)
    s1p = np.stack([tabs["s1p_re"], tabs["s1p_im"]]).astype(bf16)
    st = tabs["st128"].astype(bf16)
    tw = tabs["tw"].astype(bf16)
    maps = []
    for i in range(NCORES):
        sl = slice(i * CPC, (i + 1) * CPC)
        maps.append({
            "x_in": xf[sl].reshape(CPC, 64, 1024).astype(bf16),
            "h_in": hf[sl].reshape(CPC, 64, 1024).astype(bf16),
            "st_in": st,
            "s1_in": s1,
            "s1p_in": s1p,
            "tw_in": tw,
        })
    return maps


def kernel(x, h):
    nc, tabs = _get()
    maps = _in_maps(x, h, tabs)
    res = run_bass_kernel_spmd(nc, maps, core_ids=list(range(NCORES)))
    y = np.concatenate([r["y_out"].reshape(CPC, 65536) for r in res.results])
    return y.reshape(Bsz, Csz, T).astype(np.float32)
